# revision 9
# baseline (speedup 1.0000x reference)
"""AttentionPooling kernel for 8 Trainium2 NeuronCores.

Computation (per graph g): out[g] = sum_i softmax(logits)_i * x_i over nodes i
in g, where logits = tanh(x @ W1 + b1) @ W2 + b2.

Design (v2):
- logits are bounded (|logit| <= sum|W2| + |b2| < 17), so exp() is safe without
  max-subtraction: w_i = e_i / sum(e). numer[g, 0:256] and the denominator
  (col 256) accumulate in one PSUM bank per 128-graph block; divide at the end.
- 8192 graphs = 8 cores x 8 blocks x 128 graphs. Nodes of a block are padded to
  T_blk*128 slots; batch is sorted so blocks are contiguous (host-gathered).
- MLP input path is fp8-e4m3 (x*32, W1*2048; scales folded into the tanh
  activation scale) using DoubleRow matmuls: one matmul does the full K=256
  contraction for 512 nodes. The numer path stays bf16.
- 3-stage software pipeline over groups of 16 subtiles (G16):
    stage1(g): DMA prefetch, MLP (4 DR matmuls/unit-of-8), tanh (FD=1024)
    stage2(g-1): logit pairs (th-slices x W2, N=1), exp (FD=16), onehot (DVE)
    stage3(g-2): numer matmuls + block epilogue
  This keeps the PE stream dense: measured pair rates are stream-bound
  (LDWEIGHTS overlaps), so PE ~ 8*216 + 32*25 + 16*110 ns per 16 subtiles.
"""

import math
import os

from contextlib import ExitStack

import numpy as np
import ml_dtypes

try:
    import concourse.bass as bass
except ImportError:  # fallback if PYTHONPATH lacks the repo
    import sys

    sys.path.insert(0, "/opt/trn_rl_repo")
    import concourse.bass as bass

import concourse.tile as tile
from concourse import bass_utils, mybir

BF16 = ml_dtypes.bfloat16
E4M3 = ml_dtypes.float8_e4m3
F32 = np.float32

N_CORES = 8
N_NODES = 1_000_000
H = 256  # hidden
G = 8192  # num graphs
GPC = G // N_CORES  # graphs per core = 1024
GPB = 128  # graphs per block (= PSUM partitions)
BPC = GPC // GPB  # blocks per core = 8
P = 128  # partitions / nodes per subtile
G16 = 16  # subtiles per pipeline group

SX = 32.0  # fp8 scale for x
SW = 2048.0  # fp8 scale for W1


def _split_sync_waits(nc, maxw: int = 1) -> int:
    """The walrus build in this container rejects instructions carrying more
    than one sync-wait. Hoist extra waits onto NoOps inserted just before the
    instruction (same engine, same order => identical semantics)."""
    cnt = 0
    for f in nc.m.functions:
        for bb in f.blocks:
            insts = bb.instructions
            out = []
            changed = False
            for ins in insts:
                si = ins.sync_info
                if si is not None and len(si.on_wait) > maxw:
                    waits = list(si.on_wait)
                    keep, extra = waits[-maxw:], waits[:-maxw]
                    for w in extra:
                        cnt += 1
                        nop = mybir.InstNoOp(
                            name=f"wsplit-{cnt}",
                            engine=ins.engine,
                            sync_info=mybir.SyncInfo(on_wait=[w], on_update=[]),
                            bass_nofuse=True,
                        )
                        nc.register_instruction(nop, overwrite=True)
                        out.append(nop)
                    ins.sync_info = mybir.SyncInfo(
                        on_wait=keep, on_update=si.on_update
                    )
                    changed = True
                out.append(ins)
            if changed:
                bb.instructions = out
    return cnt


def _build_program(T_blk: int):
    nc = bass.Bass("TRN2", target_bir_lowering=False)
    T_tot = BPC * T_blk
    assert T_tot % G16 == 0
    NG = T_tot // G16  # pipeline groups
    L = T_tot * P  # node slots per core

    f32 = mybir.dt.float32
    bf16 = mybir.dt.bfloat16
    fp8e4 = mybir.dt.float8e4

    xt8_d = nc.declare_dram_parameter("xt8", [P, 2, L], fp8e4, isOutput=False)
    xn_d = nc.declare_dram_parameter("xn", [P, T_tot, H + 1], bf16, isOutput=False)
    bc_d = nc.declare_dram_parameter("bc", [P, T_tot], f32, isOutput=False)
    w1a_d = nc.declare_dram_parameter("w1a8", [P, 2, P], fp8e4, isOutput=False)
    w1b_d = nc.declare_dram_parameter("w1b8", [P, 2, P], fp8e4, isOutput=False)
    w2a_d = nc.declare_dram_parameter("w2a", [P, 1], bf16, isOutput=False)
    w2b_d = nc.declare_dram_parameter("w2b", [P, 1], bf16, isOutput=False)
    b1a_d = nc.declare_dram_parameter("b1a", [P, 1], f32, isOutput=False)
    b1b_d = nc.declare_dram_parameter("b1b", [P, 1], f32, isOutput=False)
    b2c_d = nc.declare_dram_parameter("b2c", [P, 1], f32, isOutput=False)
    iota_d = nc.declare_dram_parameter("iota", [P, P], bf16, isOutput=False)
    out_d = nc.declare_dram_parameter("out", [GPC, H], f32, isOutput=True)

    Tanh = mybir.ActivationFunctionType.Tanh
    Exp = mybir.ActivationFunctionType.Exp
    EQ = mybir.AluOpType.is_equal
    MUL = mybir.AluOpType.mult
    ADD = mybir.AluOpType.add
    DR = mybir.MatmulPerfMode.DoubleRow

    PRE = 2  # DMA prefetch depth (groups)

    with tile.TileContext(nc) as tc:
        with ExitStack() as ctx:
            consts = ctx.enter_context(tc.tile_pool(name="consts", bufs=1))
            xtpool = ctx.enter_context(tc.tile_pool(name="xt", bufs=PRE + 2))
            xnpool = ctx.enter_context(tc.tile_pool(name="xnp", bufs=PRE + 4))
            thpool = ctx.enter_context(tc.tile_pool(name="th", bufs=6))
            ohpool = ctx.enter_context(tc.tile_pool(name="oh", bufs=4))
            epool = ctx.enter_context(tc.tile_pool(name="e", bufs=3))
            outpool = ctx.enter_context(tc.tile_pool(name="outp", bufs=2))
            ps_ht = ctx.enter_context(
                tc.tile_pool(name="ps_ht", bufs=1, space=bass.MemorySpace.PSUM)
            )
            ps_lg = ctx.enter_context(
                tc.tile_pool(name="ps_lg", bufs=2, space=bass.MemorySpace.PSUM)
            )
            ps_nm = ctx.enter_context(
                tc.tile_pool(name="ps_nm", bufs=1, space=bass.MemorySpace.PSUM)
            )

            # ---- constants (loaded once) ----
            w1a_t = consts.tile([P, 2, P], fp8e4)
            nc.sync.dma_start(w1a_t[:], w1a_d[:])
            w1b_t = consts.tile([P, 2, P], fp8e4)
            nc.sync.dma_start(w1b_t[:], w1b_d[:])
            w2a_t = consts.tile([P, 1], bf16)
            nc.sync.dma_start(w2a_t[:], w2a_d[:])
            w2b_t = consts.tile([P, 1], bf16)
            nc.sync.dma_start(w2b_t[:], w2b_d[:])
            b1a_t = consts.tile([P, 1], f32)
            nc.sync.dma_start(b1a_t[:], b1a_d[:])
            b1b_t = consts.tile([P, 1], f32)
            nc.sync.dma_start(b1b_t[:], b1b_d[:])
            b2c_t = consts.tile([P, 1], f32)
            nc.sync.dma_start(b2c_t[:], b2c_d[:])
            iota_t = consts.tile([P, P], bf16)
            nc.sync.dma_start(iota_t[:], iota_d[:])
            bc_t = consts.tile([P, T_tot], f32)
            nc.sync.dma_start(bc_t[:], bc_d[:])

            MLP_SCALE = 1.0 / (SX * SW)

            xt_tiles = {}
            xn_tiles = {}
            th_tiles = {}
            e_tiles = {}
            oh_tiles = {}
            numer = [None, None]  # parity A/B accumulators

            def dma_group(g):
                xt = xtpool.tile([P, 2, G16 * P], fp8e4, tag="xt8", name="xt8")
                nc.sync.dma_start(
                    xt[:], xt8_d[:, :, g * G16 * P : (g + 1) * G16 * P]
                )
                xn = xnpool.tile([P, G16, H + 1], bf16, tag="xn", name="xn")
                nc.sync.dma_start(xn[:], xn_d[:, g * G16 : (g + 1) * G16, :])
                xt_tiles[g] = xt
                xn_tiles[g] = xn

            def stage1(g):
                xt = xt_tiles.pop(g)
                ths = []
                for u in range(2):
                    hta = ps_ht.tile([P, 2 * 512], f32, tag="hta", name="hta")
                    htb = ps_ht.tile([P, 2 * 512], f32, tag="htb", name="htb")
                    # alternate output banks so fill/drain overlap on the PE
                    for half in range(2):
                        rs = u * 1024 + half * 512
                        nc.tensor.matmul(
                            hta[:, half * 512 : (half + 1) * 512],
                            w1a_t[:],
                            xt[:, :, rs : rs + 512],
                            start=True, stop=True,
                            skip_group_check=True, perf_mode=DR,
                        )
                        nc.tensor.matmul(
                            htb[:, half * 512 : (half + 1) * 512],
                            w1b_t[:],
                            xt[:, :, rs : rs + 512],
                            start=True, stop=True,
                            skip_group_check=True, perf_mode=DR,
                        )
                    tha = thpool.tile([P, 1024], bf16, tag="tha", name="tha")
                    nc.scalar.activation(
                        tha[:], hta[:], Tanh, bias=b1a_t[:], scale=MLP_SCALE
                    )
                    thb = thpool.tile([P, 1024], bf16, tag="thb", name="thb")
                    nc.scalar.activation(
                        thb[:], htb[:], Tanh, bias=b1b_t[:], scale=MLP_SCALE
                    )
                    ths.append((tha, thb))
                th_tiles[g] = ths

            def stage23(g2, g3):
                """Interleave logit pairs (group g2) with numer matmuls
                (group g3) so consecutive PE matmuls hit different PSUM
                banks and fill/drain overlap."""
                ths = th_tiles.pop(g2) if g2 is not None else None
                if ths is not None:
                    lg = ps_lg.tile([P, G16], f32, tag="lg", name="lg")
                ohs3 = oh_tiles.pop(g3) if g3 is not None else None
                xn3 = xn_tiles.pop(g3) if g3 is not None else None
                for i in range(G16):
                    if ths is not None:
                        u, s8 = divmod(i, 8)
                        tha, thb = ths[u]
                        nc.tensor.matmul(
                            lg[:, i : i + 1],
                            tha[:, s8 * P : (s8 + 1) * P],
                            w2a_t[:],
                            start=True, stop=False, skip_group_check=True,
                        )
                        nc.tensor.matmul(
                            lg[:, i : i + 1],
                            thb[:, s8 * P : (s8 + 1) * P],
                            w2b_t[:],
                            start=False, stop=True, skip_group_check=True,
                        )
                    if ohs3 is not None:
                        numer_mm(g3, i, ohs3, xn3)
                if ths is not None:
                    ec = epool.tile([P, G16], f32, tag="ec", name="ec")
                    nc.scalar.activation(ec[:], lg[:], Exp, bias=b2c_t[:])
                    ohs = []
                    for i in range(G16):
                        j = g2 * G16 + i
                        oh = ohpool.tile(
                            [P, P], bf16, tag=f"oh{i}", name=f"oh{i}"
                        )
                        nc.vector.tensor_scalar(
                            oh[:], iota_t[:], bc_t[:, j : j + 1],
                            ec[:, i : i + 1], EQ, MUL,
                        )
                        ohs.append(oh)
                    oh_tiles[g2] = ohs

            def numer_mm(g, i, ohs, xn):
                # two parity accumulators in different PSUM banks so
                # consecutive numer matmuls overlap fill/drain
                j = g * G16 + i
                blk, t_in_blk = divmod(j, T_blk)
                par = t_in_blk % 2
                if t_in_blk == 0:
                    numer[0] = ps_nm.tile([P, H + 1], f32, tag="nmA", name="nmA")
                    numer[1] = ps_nm.tile([P, H + 1], f32, tag="nmB", name="nmB")
                nc.tensor.matmul(
                    numer[par][:],
                    ohs[i][:],
                    xn[:, i, :],
                    start=(t_in_blk < 2),
                    stop=(t_in_blk >= T_blk - 2),
                    skip_group_check=True,
                )
                if t_in_blk == T_blk - 1:
                    nmA, nmB = numer
                    smA = outpool.tile([P, H + 1], f32, tag="smA", name="smA")
                    nc.vector.tensor_copy(smA[:], nmA[:])
                    sm = outpool.tile([P, H + 1], f32, tag="sm", name="sm")
                    nc.vector.tensor_add(sm[:], smA[:], nmB[:])
                    dn = epool.tile([P, 1], f32, tag="dn", name="dn")
                    nc.vector.tensor_scalar(
                        dn[:], sm[:, H : H + 1], 1e-30, None, ADD
                    )
                    rec = epool.tile([P, 1], f32, tag="rec", name="rec")
                    nc.vector.reciprocal(rec[:], dn[:])
                    outt = outpool.tile([P, H], f32, tag="outt", name="outt")
                    nc.vector.tensor_scalar(
                        outt[:], sm[:, 0:H], rec[:], None, MUL
                    )
                    nc.sync.dma_start(
                        out_d[blk * GPB : (blk + 1) * GPB, :], outt[:]
                    )

            for g in range(min(PRE, NG)):
                dma_group(g)
            for g in range(NG + 2):
                if g + PRE < NG:
                    dma_group(g + PRE)
                if g < NG:
                    stage1(g)
                g2 = g - 1 if 0 <= g - 1 < NG else None
                g3 = g - 2 if 0 <= g - 2 < NG else None
                if g2 is not None or g3 is not None:
                    stage23(g2, g3)

    return nc


def _run_warmup():
    """Run a tiny NEFF touching every engine/op first. The first NEFF executed
    in a fresh process has been observed to hang when it contains the full
    pipeline (ACT table staging race?); a small warmup run avoids it."""
    f32 = mybir.dt.float32
    Tanh = mybir.ActivationFunctionType.Tanh
    Exp = mybir.ActivationFunctionType.Exp
    EQ = mybir.AluOpType.is_equal
    MUL = mybir.AluOpType.mult
    nc = bass.Bass("TRN2", target_bir_lowering=False)
    x_d = nc.declare_dram_parameter("x", [P, P], f32, isOutput=False)
    y_d = nc.declare_dram_parameter("y", [P, P], f32, isOutput=True)
    with tile.TileContext(nc) as tc:
        with ExitStack() as ctx:
            pool = ctx.enter_context(tc.tile_pool(name="p", bufs=2))
            ps = ctx.enter_context(
                tc.tile_pool(name="ps", bufs=1, space=bass.MemorySpace.PSUM)
            )
            t = pool.tile([P, P], f32)
            nc.sync.dma_start(t[:], x_d[:])
            acc = ps.tile([P, P], f32)
            nc.tensor.matmul(acc[:], t[:], t[:], start=True, stop=True)
            t2 = pool.tile([P, P], f32)
            nc.scalar.activation(t2[:], acc[:], Tanh, bias=t[:, 0:1])
            t3 = pool.tile([P, P], f32)
            nc.scalar.activation(t3[:], t2[:], Exp, bias=t[:, 0:1])
            t4 = pool.tile([P, P], f32)
            nc.vector.tensor_scalar(t4[:], t3[:], t[:, 0:1], t[:, 1:2], EQ, MUL)
            t5 = pool.tile([P, 1], f32)
            nc.vector.reciprocal(t5[:], t3[:, 0:1])
            nc.vector.tensor_scalar(t4[:, 0:1], t5[:], t5[:], None, MUL)
            nc.sync.dma_start(y_d[:], t4[:])
    _split_sync_waits(nc)
    xw = np.zeros((P, P), np.float32)
    bass_utils.run_bass_kernel_spmd(
        nc, [{"x": xw} for _ in range(N_CORES)], list(range(N_CORES))
    )


def prepare_inputs(x, batch, W1, b1, W2, b2):
    """Host-side segmentation + per-core gather. Returns (T_blk, in_maps)."""
    import time as _time

    x = np.asarray(x, dtype=F32)
    batch = np.asarray(batch).astype(np.int64)
    W1 = np.asarray(W1, dtype=F32)
    b1 = np.asarray(b1, dtype=F32)
    W2 = np.asarray(W2, dtype=F32)
    b2 = np.asarray(b2, dtype=F32)
    assert x.shape == (N_NODES, H) and batch.shape == (N_NODES,)

    block_starts = np.searchsorted(batch, np.arange(0, G + 1, GPB)).astype(np.int64)
    cnts = np.diff(block_starts)
    T_blk = max(1, int(math.ceil(cnts.max() / P)))
    if T_blk % 2:
        T_blk += 1  # T_tot = 8*T_blk must be a multiple of G16=16
    T_tot = BPC * T_blk
    L = T_tot * P

    _tg = _time.time()
    in_maps = []
    for c in range(N_CORES):
        xn_c = np.zeros((P, T_tot, H + 1), dtype=BF16)
        xn_c[:, :, H] = F32(1.0)
        xt8_c = np.zeros((P, 2, L), dtype=E4M3)
        bc_c = np.full((P, T_tot), -1.0, dtype=F32)
        for b in range(BPC):
            gblk = c * BPC + b
            s = int(block_starts[gblk])
            e = min(s + T_blk * P, N_NODES)
            n = e - s
            if n <= 0:
                continue
            seg = x[s:e]
            nb = (n + P - 1) // P  # subtiles actually used
            pad = np.zeros((nb * P, H), dtype=F32)
            pad[:n] = seg
            # xn: [p, t, h] with node slot t*128+p
            xn_c[:, b * T_blk : b * T_blk + nb, 0:H] = pad.reshape(
                nb, P, H
            ).transpose(1, 0, 2)
            # xt8: [p, ktile, slot] = x[slot, ktile*128+p] * SX in e4m3
            xq = (pad.T * SX).astype(E4M3)  # [256, nb*P]
            r0 = b * T_blk * P
            xt8_c[:, :, r0 : r0 + nb * P] = xq.reshape(2, P, nb * P).transpose(
                1, 0, 2
            )
            vals = np.full(T_blk * P, -1.0, dtype=F32)
            vals[:n] = (batch[s:e] - gblk * GPB).astype(F32)
            bc_c[:, b * T_blk : (b + 1) * T_blk] = vals.reshape(T_blk, P).T
        in_maps.append({"xt8": xt8_c, "xn": xn_c, "bc": bc_c})
    print(f"[kernel] host gather: {_time.time()-_tg:.1f}s", flush=True)

    W1q = (W1 * SW).astype(E4M3)  # [256, 256]
    w1r = W1q.reshape(2, P, H).transpose(1, 0, 2)  # [p, t, m]
    consts = {
        "w1a8": np.ascontiguousarray(w1r[:, :, 0:P]),
        "w1b8": np.ascontiguousarray(w1r[:, :, P:H]),
        "w2a": W2[0:P, :].astype(BF16),
        "w2b": W2[P:H, :].astype(BF16),
        "b1a": b1[0:P, None].astype(F32),
        "b1b": b1[P:H, None].astype(F32),
        "b2c": np.full((P, 1), b2[0] if b2.ndim else b2, dtype=F32),
        "iota": np.tile(np.arange(P, dtype=F32), (P, 1)).astype(BF16),
    }
    for m in in_maps:
        m.update(consts)
    return T_blk, in_maps


LAST_EXEC_NS = None


def kernel(x, batch, num_graphs, W1, b1, W2, b2):
    import time as _time

    global LAST_EXEC_NS
    ng = int(num_graphs)
    assert ng == G
    T_blk, in_maps = prepare_inputs(x, batch, W1, b1, W2, b2)

    t0 = _time.time()
    nc = _build_program(T_blk)
    _split_sync_waits(nc)
    print(f"[kernel] build+split: {_time.time()-t0:.1f}s (T_blk={T_blk})", flush=True)

    t0 = _time.time()
    _run_warmup()
    print(f"[kernel] warmup run: {_time.time()-t0:.1f}s", flush=True)

    trace = os.environ.get("KERNEL_TRACE", "0") == "1"
    tmpdir = os.environ.get("KERNEL_TRACE_DIR") or None
    if tmpdir:
        os.makedirs(tmpdir, exist_ok=True)

    t0 = _time.time()
    res = bass_utils.run_bass_kernel_spmd(
        nc, in_maps, list(range(N_CORES)), trace=trace, tmpdir=tmpdir
    )
    print(f"[kernel] main run (compile+upload+exec): {_time.time()-t0:.1f}s", flush=True)
    LAST_EXEC_NS = res.exec_time_ns

    out = np.concatenate([res.results[c]["out"] for c in range(N_CORES)], axis=0)
    return out.astype(F32)
